# revision 1
# baseline (speedup 1.0000x reference)
"""Causal ConvTranspose1d (grouped, stride 8) Trainium2 Bass kernel.

Problem (hardcoded):
  x      [8, 512, 4096]  f32
  weight [512, 16, 1]    f32
  bias   [256]           f32
  out    [8, 256, 32768] f32   (= [B, Cout, T*stride])

Math (derived from the reference grouped dilated conv):
  with w2 = weight.reshape(512, 16), cpg = 2, stride = 8, K = 16:
  y[b, co, 8*t + r] = sum_{j in 0..1} ( w2[2co+j, r]   * x[b, 2co+j, t]
                                      + w2[2co+j, r+8] * x[b, 2co+j, t-1] )
                      + bias[co]          (x[., -1] == 0)

Sharding: data-parallel over batch; one batch element per NeuronCore (8 cores).

Per-core implementation:
  - x is loaded as two SBUF tiles per half of the output channels (ct in
    {0,1}): x0 = even input channels, x1 = odd input channels of that half
    ([128, 1+4096] each, leading zero column for the t-1 tap).
  - The 4 MAC terms per output are computed on the TensorEngine as 4
    accumulating matmuls per output phase r with *diagonal* stationary
    matrices diag(w2[2c+j, k]) (K=128, M=128, fp32r), accumulated in PSUM.
  - PSUM [128co, 512t] phase tiles are interleaved into the output layout
    y[co, 8t+r] by strided-write copies that also add the bias
    (ScalarE activation-Identity / VectorE tensor_scalar_add, split).
  - [128, 4096] output tiles are DMAed back contiguously.
"""

import numpy as np

B, CIN, COUT, K, T = 8, 512, 256, 16, 4096
STRIDE = 8
SOUT = T * STRIDE  # 32768
NCORES = 8
VARIANT = "diag"

_CACHE = {}


def _build_nc(repeat=1, hw_loop=False, variant=None):
    import concourse.mybir as mybir
    from concourse import bacc
    from concourse.tile import TileContext
    from contextlib import nullcontext

    f32 = mybir.dt.float32
    f32r = mybir.dt.float32r
    variant = variant or VARIANT

    # Bacc (not plain Bass): its compile() legalizes semaphore waits for the
    # TRN2 "at most 1 wait per instruction" constraint.
    nc = bacc.Bacc(trn_type="TRN2", target_bir_lowering=False, debug=False)
    # x and the weight matrices are consumed by fp32r matmuls; walrus requires
    # the producing instructions (DMAs here) to carry the fp32r dtype.
    # x is passed host-padded with a leading zero column (for the t-1 tap).
    WDCOLS = 64 * 128 if variant.startswith("diag") else 4 * 16 * 64
    x = nc.dram_tensor("x", [CIN, 1 + T], f32r, kind="ExternalInput").ap()
    wd = nc.dram_tensor("wd", [128, WDCOLS], f32r, kind="ExternalInput").ap()
    bias = nc.dram_tensor("bias", [128, 2], f32, kind="ExternalInput").ap()
    y = nc.dram_tensor("y", [COUT, SOUT], f32, kind="ExternalOutput").ap()

    TWIN = 512          # t-window per PSUM bank (fp32 bank limit)
    NTWIN = T // TWIN   # 8

    with TileContext(nc) as tc:
        with (
            tc.tile_pool(name="const", bufs=1) as cpool,
            tc.tile_pool(name="xp", bufs=2) as xpool,
            tc.tile_pool(name="yp", bufs=3) as ypool,
            tc.tile_pool(name="ps", bufs=2, space="PSUM") as pspool,
        ):
            wd_t = cpool.tile([128, WDCOLS], f32r)
            nc.sync.dma_start(out=wd_t, in_=wd)
            bias_t = cpool.tile([128, 2], f32)
            nc.sync.dma_start(out=bias_t, in_=bias)

            def emit_pass(ct):
                # Load the x channels feeding co block ct:
                #  diag:   stride-2 slices (even/odd channels), partition = co
                #  banded: two natural 128-channel slices, partition = channel
                xj = []
                for j in range(2):
                    x_t = xpool.tile(
                        [128, 1 + T], f32r, tag=f"x{j}", name=f"x_t{j}"
                    )
                    if variant.startswith("diag"):
                        src = x[256 * ct + j : 256 * ct + 256 : 2, :]
                    else:
                        q = 2 * ct + j
                        src = x[128 * q : 128 * (q + 1), :]
                    nc.sync.dma_start(out=x_t, in_=src)
                    xj.append(x_t)

                if variant == "diag2":
                    # Twin-paired order: each diag matrix streams two
                    # consecutive t-windows back-to-back (same stationary),
                    # letting the PE overlap/skip half the weight loads.
                    for tp in range(NTWIN // 2):
                        y_ts = []
                        for w_ in range(2):
                            y_ts.append(
                                ypool.tile(
                                    [128, STRIDE * TWIN], f32, tag=f"y{w_}",
                                    name=f"y_t{w_}", bufs=2,
                                )
                            )
                        for half in range(2):
                            for i in range(4):
                                r = half * 4 + i
                                p_ts = [
                                    pspool.tile(
                                        [128, TWIN], f32, tag=f"ps{i}_{w_}",
                                        name=f"p_t{i}_{w_}", bufs=1,
                                    )
                                    for w_ in range(2)
                                ]
                                for j in range(2):
                                    for tap in range(2):
                                        k = r + 8 * tap
                                        col = ((ct * 2 + j) * 16 + k) * 128
                                        for w_ in range(2):
                                            t0 = (2 * tp + w_) * TWIN
                                            rhs = xj[j][
                                                :,
                                                (1 - tap) + t0 : (1 - tap)
                                                + t0
                                                + TWIN,
                                            ]
                                            nc.tensor.matmul(
                                                p_ts[w_],
                                                wd_t[:, col : col + 128],
                                                rhs,
                                                start=(tap == 0 and j == 0),
                                                stop=(tap == 1 and j == 1),
                                            )
                                for w_ in range(2):
                                    out_ap = y_ts[w_][
                                        :, r : STRIDE * TWIN : STRIDE
                                    ]
                                    b_ap = bias_t[:, ct : ct + 1]
                                    if r % 2 == 0:
                                        nc.scalar.add(out_ap, p_ts[w_], b_ap)
                                    else:
                                        nc.vector.tensor_scalar_add(
                                            out_ap, p_ts[w_], b_ap
                                        )
                        for w_ in range(2):
                            t0 = (2 * tp + w_) * TWIN
                            nc.sync.dma_start(
                                out=y[
                                    128 * ct : 128 * (ct + 1),
                                    STRIDE * t0 : STRIDE * t0 + STRIDE * TWIN,
                                ],
                                in_=y_ts[w_],
                            )
                    return

                for twin in range(NTWIN):
                    t0 = twin * TWIN
                    y_t = ypool.tile(
                        [128, STRIDE * TWIN], f32, tag="y", name="y_t"
                    )
                    for half in range(2):
                        for i in range(4):
                            r = half * 4 + i
                            p_t = pspool.tile(
                                [128, TWIN], f32, tag=f"ps{i}", name=f"p_t{i}"
                            )
                            mm1 = variant == "diag_mm1"
                            for j in range(1 if mm1 else 2):
                                for tap in range(1 if mm1 else 2):
                                    k = r + 8 * tap
                                    rhs = xj[j][
                                        :, (1 - tap) + t0 : (1 - tap) + t0 + TWIN
                                    ]
                                    if variant.startswith("diag"):
                                        col = ((ct * 2 + j) * 16 + k) * 128
                                        nc.tensor.matmul(
                                            p_t,
                                            wd_t[:, col : col + 128],
                                            rhs,
                                            start=(tap == 0 and j == 0),
                                            stop=mm1 or (tap == 1 and j == 1),
                                        )
                                    else:
                                        q = 2 * ct + j
                                        col = (q * 16 + k) * 64
                                        nc.tensor.matmul(
                                            p_t[64 * j : 64 * (j + 1), :],
                                            wd_t[:, col : col + 64],
                                            rhs,
                                            start=(tap == 0),
                                            stop=(tap == 1),
                                        )
                            # Interleave + bias: y_t[:, 8*t + r] = p_t[:, t] + bias
                            if variant != "diag_noint":
                                out_ap = y_t[:, r : STRIDE * TWIN : STRIDE]
                                b_ap = bias_t[:, ct : ct + 1]
                                if r % 2 == 0:
                                    nc.scalar.add(out_ap, p_t, b_ap)
                                else:
                                    nc.vector.tensor_scalar_add(out_ap, p_t, b_ap)
                    nc.sync.dma_start(
                        out=y[
                            128 * ct : 128 * (ct + 1),
                            STRIDE * t0 : STRIDE * t0 + STRIDE * TWIN,
                        ],
                        in_=y_t,
                    )

            if hw_loop:
                with tc.For_i(0, repeat, 1, name="rep"):
                    for ct in range(2):
                        emit_pass(ct)
            else:
                for _rep in range(repeat):
                    for ct in range(2):
                        emit_pass(ct)
    nc.compile()
    return nc


def _prep_weights(weight: np.ndarray, variant=None) -> np.ndarray:
    variant = variant or VARIANT
    w2 = weight.reshape(CIN, K).astype(np.float32)
    p = np.arange(128)
    if variant == "diag":
        wd = np.zeros((128, 64 * 128), np.float32)
        for ct in range(2):
            for j in range(2):
                for k in range(K):
                    base = ((ct * 2 + j) * 16 + k) * 128
                    wd[p, base + p] = w2[256 * ct + 2 * p + j, k]
    else:
        wd = np.zeros((128, 4 * 16 * 64), np.float32)
        for q in range(4):
            for k in range(K):
                base = (q * 16 + k) * 64
                wd[p, base + p // 2] = w2[128 * q + p, k]
    return wd


def _make_exec(nc):
    """Build a jitted 8-core SPMD callable for a Bass module.

    Mirrors concourse.bass2jax.run_bass_via_pjrt, but returns the jitted
    function (and zero output buffers) so repeated calls do not
    re-lower/re-compile.
    """
    import jax
    import concourse.mybir as mybir
    from concourse import bass2jax
    from jax.sharding import Mesh, PartitionSpec
    from jax.experimental.shard_map import shard_map

    bass2jax.install_neuronx_cc_hook()

    partition_name = nc.partition_id_tensor.name if nc.partition_id_tensor else None

    in_names = []
    out_names = []
    out_avals = []
    zero_outs = []
    for alloc in nc.m.functions[0].allocations:
        if not isinstance(alloc, mybir.MemoryLocationSet):
            continue
        name = alloc.memorylocations[0].name
        if alloc.kind == "ExternalInput":
            if name != partition_name:
                in_names.append(name)
        elif alloc.kind == "ExternalOutput":
            shape = tuple(alloc.tensor_shape)
            dtype = mybir.dt.np(alloc.dtype)
            out_names.append(name)
            out_avals.append(jax.core.ShapedArray(shape, dtype))
            zero_outs.append(np.zeros(shape, dtype))
    n_params = len(in_names)
    all_in_names = list(in_names) + list(out_names)
    if partition_name is not None:
        all_in_names.append(partition_name)

    def _body(*args):
        operands = list(args)
        if partition_name is not None:
            operands.append(bass2jax.partition_id_tensor())
        outs = bass2jax._bass_exec_p.bind(
            *operands,
            out_avals=tuple(out_avals),
            in_names=tuple(all_in_names),
            out_names=tuple(out_names),
            lowering_input_output_aliases=(),
            sim_require_finite=True,
            sim_require_nnan=True,
            nc=nc,
        )
        return tuple(outs)

    devices = jax.devices()[:NCORES]
    mesh = Mesh(np.asarray(devices), ("core",))
    n_outs = len(out_names)
    in_specs = (PartitionSpec("core"),) * (n_params + n_outs)
    out_specs = (PartitionSpec("core"),) * n_outs
    sharded = jax.jit(
        shard_map(
            _body, mesh=mesh, in_specs=in_specs, out_specs=out_specs, check_rep=False
        ),
        keep_unused=True,
    )
    concat_zeros = [
        np.zeros((NCORES * z.shape[0], *z.shape[1:]), z.dtype) for z in zero_outs
    ]
    return (sharded, in_names, out_names, out_avals, concat_zeros)


def _get_exec():
    if "exec" not in _CACHE:
        nc = _build_nc()
        _CACHE["nc"] = nc
        _CACHE["exec"] = _make_exec(nc)
    return _CACHE["exec"]


def _make_concat_inputs(x, weight, bias):
    """Per-core input dict -> concatenated global arrays (order = in_names)."""
    wd = _prep_weights(weight)
    bias2 = np.stack([bias[:128], bias[128:]], axis=1).astype(np.float32)
    xp = np.zeros((NCORES, CIN, 1 + T), np.float32)
    xp[:, :, 1:] = x
    per_core = {
        "x": xp.reshape(NCORES * CIN, 1 + T),
        "wd": np.concatenate([wd] * NCORES, axis=0),
        "bias": np.concatenate([bias2] * NCORES, axis=0),
    }
    return per_core


def kernel(x, weight, bias) -> np.ndarray:
    x = np.asarray(x, dtype=np.float32)
    weight = np.asarray(weight, dtype=np.float32)
    bias = np.asarray(bias, dtype=np.float32)

    sharded, in_names, out_names, out_avals, concat_zeros = _get_exec()
    per_core = _make_concat_inputs(x, weight, bias)
    concat_in = [per_core[name] for name in in_names]
    out_arrs = sharded(*concat_in, *concat_zeros)
    yi = out_names.index("y")
    out = np.asarray(out_arrs[yi]).reshape(NCORES, COUT, SOUT)
    return out.astype(np.float32)

